# revision 5
# baseline (speedup 1.0000x reference)
"""
MultiHeadDotProductAttention (B=4, S=2048, E=2048, H=16, Dh=128) on 8 trn2 cores.

Sharding: core c -> (batch b = c//2, head-group g = c%2 of 8 heads).
Each core computes the full attention + output projection for its 8 heads on its
batch; the two head-group partial outputs per batch are summed on the host
(output projection is linear over heads).

Per-core pipeline (all matmul operands fp16 except exp(logits)/v in bf16,
fp32 PSUM accumulation everywhere):
  A:  qT_h[d, s]   = (Wq_h^T Xq^T), RMS-normed over d via ones-matmul partition
      reduction, scaled by q_ln_scale (per-partition) -> fp16
  B1: kT_h[d, s]   same from Xkv / Wk / k_ln_scale
  B2: v[s, hd]     plain projection -> bf16
  C:  per (q-block of 512, head):
        S^T[k, q]  = kT_h^T qT_h  (one matmul per 128-k tile, no accumulation)
        E = exp(S^T) (no max subtraction: max logit ~71 << 88, fits fp32/bf16)
        ctx^T[d, q] += v_tile^T E_tile  (PSUM accumulate over 16 k-tiles)
        rowsum(E) over k: DVE pairwise tree over k-tiles + f32 ones-matmul
        ctx^T * (1/sum) broadcast -> fp16 (normalization fused into eviction)
  D:  out[q, e]    = sum_h ctx_h ctx^T-as-lhsT @ Wo_h  (PSUM accumulate over h)
"""

import numpy as np

B, S, E, H, Dh = 4, 2048, 2048, 16, 128
HG = H // 2            # heads per core
P = 128
ET = E // P            # contraction tiles for projections
KT = S // P            # key tiles
QB = S // 512          # 512-wide query blocks
EPS = 1e-6
N_CORES = 8

_cached = None


def _build_program():
    import concourse.bass as bass
    import concourse.tile as tile
    from concourse import bacc, mybir

    f32 = mybir.dt.float32
    f16 = mybir.dt.float16
    bf16 = mybir.dt.bfloat16
    MUL = mybir.AluOpType.mult
    ADD = mybir.AluOpType.add
    Act = mybir.ActivationFunctionType

    nc = bacc.Bacc("TRN2", target_bir_lowering=False, debug=False,
                   num_devices=N_CORES)

    xqT = nc.dram_tensor("xqT", [E, S], f16, kind="ExternalInput").ap()
    xkT = nc.dram_tensor("xkT", [E, S], f16, kind="ExternalInput").ap()
    wq = nc.dram_tensor("wq", [E, HG * Dh], f16, kind="ExternalInput").ap()
    wk = nc.dram_tensor("wk", [E, HG * Dh], f16, kind="ExternalInput").ap()
    wv = nc.dram_tensor("wv", [E, HG * Dh], f16, kind="ExternalInput").ap()
    wo = nc.dram_tensor("wo", [HG * Dh, E], f16, kind="ExternalInput").ap()
    qs = nc.dram_tensor("qs", [Dh, 1], f32, kind="ExternalInput").ap()
    ks = nc.dram_tensor("ks", [Dh, 1], f32, kind="ExternalInput").ap()
    out = nc.dram_tensor("out", [S, E], f32, kind="ExternalOutput").ap()

    from contextlib import ExitStack

    from concourse import library_config

    with tile.TileContext(nc) as tc, ExitStack() as top:
        nc.gpsimd.load_library(library_config.attnmlp)
        const = top.enter_context(tc.tile_pool(name="const", bufs=1))
        ones = const.tile([P, 1], f32)
        nc.vector.memset(ones, 1.0)
        ones16 = const.tile([P, 1], f16)
        nc.vector.memset(ones16, 1.0)
        qs_sb = const.tile([P, 1], f32)
        nc.sync.dma_start(out=qs_sb, in_=qs)
        ks_sb = const.tile([P, 1], f32)
        nc.sync.dma_start(out=ks_sb, in_=ks)
        eps_t = const.tile([1, 1], f32)
        nc.vector.memset(eps_t, EPS)

        persist = top.enter_context(tc.tile_pool(name="persist", bufs=1))
        qTn = persist.tile([P, HG * S], f16)     # per head: [d, s]
        kTn = persist.tile([P, HG * S], f16)
        v_sb = persist.tile([P, KT * HG * Dh], bf16)  # 16 s-tiles x [128, 1024]
        ctxn = persist.tile([P, HG * S], f16)    # per head: [d, q]

        def proj_norm(xT_ap, w_ap, scale_sb, dst):
            """Compute RMS-normed (W_h^T X^T) for all 8 heads -> dst[d, h*S+s]."""
            with ExitStack() as ph:
                wpool = ph.enter_context(tc.tile_pool(name="w", bufs=1))
                w_sb = wpool.tile([P, ET * HG * Dh], f16)
                for et in range(ET):
                    nc.sync.dma_start(
                        out=w_sb[:, et * 1024:(et + 1) * 1024],
                        in_=w_ap[et * P:(et + 1) * P, :])
                xpool = ph.enter_context(tc.tile_pool(name="xblk", bufs=2))
                pps = ph.enter_context(
                    tc.tile_pool(name="pj_ps", bufs=4, space="PSUM"))
                sqp = ph.enter_context(tc.tile_pool(name="sq", bufs=3))
                smp = ph.enter_context(tc.tile_pool(name="sm", bufs=3))
                rp = ph.enter_context(tc.tile_pool(name="R", bufs=3))
                for qb in range(QB):
                    xblk = xpool.tile([P, ET * 512], f16, tag="xblk")
                    for et in range(ET):
                        nc.sync.dma_start(
                            out=xblk[:, et * 512:(et + 1) * 512],
                            in_=xT_ap[et * P:(et + 1) * P,
                                      qb * 512:(qb + 1) * 512])
                    for h in range(HG):
                        ps = pps.tile([P, 512], f32, tag="pj")
                        for et in range(ET):
                            nc.tensor.matmul(
                                ps,
                                lhsT=w_sb[:, et * 1024 + h * Dh:
                                          et * 1024 + (h + 1) * Dh],
                                rhs=xblk[:, et * 512:(et + 1) * 512],
                                start=(et == 0), stop=(et == ET - 1))
                        sq = sqp.tile([P, 512], f16, tag="sq")
                        nc.scalar.activation(sq, ps, Act.Square)
                        ssq = pps.tile([1, 512], f32, tag="pj")
                        nc.tensor.matmul(ssq, lhsT=ones16[:, :1], rhs=sq,
                                         start=True, stop=True)
                        rms = smp.tile([1, 512], f32, tag="sm")
                        nc.scalar.activation(rms, ssq, Act.Sqrt,
                                             bias=eps_t[:, :1], scale=1.0 / Dh)
                        rinv = smp.tile([1, 512], f32, tag="sm")
                        nc.vector.reciprocal(rinv, rms)
                        R = rp.tile([P, 512], f32, tag="R")
                        nc.gpsimd.partition_broadcast(R, rinv)
                        Rs = rp.tile([P, 512], f32, tag="R")
                        nc.vector.tensor_scalar_mul(Rs, R, scale_sb[:, :1])
                        nc.vector.tensor_tensor(
                            dst[:, h * S + qb * 512: h * S + (qb + 1) * 512],
                            ps, Rs, MUL)

        with nc.named_scope("phaseA_q"):
            proj_norm(xqT, wq, qs_sb, qTn)
        with nc.named_scope("phaseB1_k"):
            proj_norm(xkT, wk, ks_sb, kTn)

        with nc.named_scope("phaseB2_v"), ExitStack() as ph:
            wpool = ph.enter_context(tc.tile_pool(name="wv", bufs=1))
            wv_sb = wpool.tile([P, ET * HG * Dh], f16)
            for et in range(ET):
                nc.sync.dma_start(
                    out=wv_sb[:, et * 1024:(et + 1) * 1024],
                    in_=wv[et * P:(et + 1) * P, :])
            xpool = ph.enter_context(tc.tile_pool(name="xblkv", bufs=2))
            pps = ph.enter_context(
                tc.tile_pool(name="v_ps", bufs=4, space="PSUM"))
            for sb in range(QB):
                xblk = xpool.tile([P, ET * 512], f16, tag="xblkv")
                for et in range(ET):
                    nc.sync.dma_start(
                        out=xblk[:, et * 512:(et + 1) * 512],
                        in_=xkT[et * P:(et + 1) * P, sb * 512:(sb + 1) * 512])
                for ss in range(4):
                    st = sb * 4 + ss      # s-tile index 0..15
                    for hb in range(2):   # 512-wide hd blocks
                        ps = pps.tile([P, 512], f32, tag="v")
                        for et in range(ET):
                            nc.tensor.matmul(
                                ps,
                                lhsT=xblk[:, et * 512 + ss * P:
                                          et * 512 + (ss + 1) * P],
                                rhs=wv_sb[:, et * 1024 + hb * 512:
                                          et * 1024 + (hb + 1) * 512],
                                start=(et == 0), stop=(et == ET - 1))
                        nc.scalar.copy(
                            v_sb[:, st * 1024 + hb * 512:
                                 st * 1024 + (hb + 1) * 512], ps)

        with nc.named_scope("phaseC_attn"), ExitStack() as ph:
            psS = ph.enter_context(
                tc.tile_pool(name="S_ps", bufs=3, space="PSUM"))
            psC = ph.enter_context(
                tc.tile_pool(name="ctx_ps", bufs=2, space="PSUM"))
            epool = ph.enter_context(tc.tile_pool(name="E", bufs=6))
            tpool = ph.enter_context(tc.tile_pool(name="tree", bufs=8))
            smp = ph.enter_context(tc.tile_pool(name="smC", bufs=3))
            rp = ph.enter_context(tc.tile_pool(name="RC", bufs=3))
            for qb in range(QB):
                for h in range(HG):
                    ctx_ps = psC.tile([P, 512], f32, tag="ctx")
                    stack = []  # (level, f32 partial-sum tile)

                    def push(lvl, t):
                        while stack and stack[-1][0] == lvl:
                            _, o = stack.pop()
                            nt = tpool.tile([P, 512], f32, tag="tree")
                            nc.vector.tensor_tensor(nt, o, t, ADD)
                            t, lvl = nt, lvl + 1
                        stack.append((lvl, t))

                    for kg in range(KT // 2):
                        s_ps = psS.tile([P, 1024], f32, tag="S")
                        for j in range(2):
                            kt = kg * 2 + j
                            nc.tensor.matmul(
                                s_ps[:, j * 512:(j + 1) * 512],
                                lhsT=kTn[:, h * S + kt * P:
                                         h * S + (kt + 1) * P],
                                rhs=qTn[:, h * S + qb * 512:
                                        h * S + (qb + 1) * 512],
                                start=True, stop=True)
                        e_t = epool.tile([P, 1024], bf16, tag="E")
                        nc.scalar.activation(e_t, s_ps, Act.Exp)
                        for j in range(2):
                            kt = kg * 2 + j
                            nc.tensor.matmul(
                                ctx_ps,
                                lhsT=v_sb[:, kt * 1024 + h * Dh:
                                          kt * 1024 + (h + 1) * Dh],
                                rhs=e_t[:, j * 512:(j + 1) * 512],
                                start=(kt == 0), stop=(kt == KT - 1))
                        lt = tpool.tile([P, 512], f32, tag="tree")
                        nc.vector.tensor_tensor(
                            lt, e_t[:, :512], e_t[:, 512:], ADD)
                        push(1, lt)
                    while len(stack) > 1:
                        _, a = stack.pop()
                        _, b_ = stack.pop()
                        nt = tpool.tile([P, 512], f32, tag="tree")
                        nc.vector.tensor_tensor(nt, b_, a, ADD)
                        stack.append((99, nt))
                    sum_f = stack.pop()[1]
                    ssum = psS.tile([1, 512], f32, tag="S")
                    nc.tensor.matmul(ssum, lhsT=ones[:, :1], rhs=sum_f,
                                     start=True, stop=True)
                    srec = smp.tile([1, 512], f32, tag="smC")
                    nc.vector.reciprocal(srec, ssum)
                    Rn = rp.tile([P, 512], f32, tag="RC")
                    nc.gpsimd.partition_broadcast(Rn, srec)
                    nc.vector.tensor_tensor(
                        ctxn[:, h * S + qb * 512: h * S + (qb + 1) * 512],
                        ctx_ps, Rn, MUL)

        with nc.named_scope("phaseD_out"), ExitStack() as ph:
            wop = ph.enter_context(tc.tile_pool(name="wo", bufs=2))
            psD = ph.enter_context(
                tc.tile_pool(name="o_ps", bufs=4, space="PSUM"))
            obuf = ph.enter_context(tc.tile_pool(name="obuf", bufs=4))
            for eb in range(4):
                wo_sb = wop.tile([P, HG * 512], f16, tag="wo")
                for h in range(HG):
                    nc.sync.dma_start(
                        out=wo_sb[:, h * 512:(h + 1) * 512],
                        in_=wo[h * Dh:(h + 1) * Dh,
                               eb * 512:(eb + 1) * 512])
                for qt in range(KT):
                    ops = psD.tile([P, 512], f32, tag="o")
                    for h in range(HG):
                        nc.tensor.matmul(
                            ops,
                            lhsT=ctxn[:, h * S + qt * P: h * S + (qt + 1) * P],
                            rhs=wo_sb[:, h * 512:(h + 1) * 512],
                            start=(h == 0), stop=(h == HG - 1))
                    osb = obuf.tile([P, 512], f32, tag="ob")
                    nc.vector.tensor_copy(osb, ops)
                    nc.sync.dma_start(
                        out=out[qt * P:(qt + 1) * P, eb * 512:(eb + 1) * 512],
                        in_=osb)
    nc.compile()
    return nc


def _get_program():
    global _cached
    if _cached is None:
        _cached = _build_program()
    return _cached


def _make_in_maps(inputs_q, inputs_kv, Wq, Wk, Wv, q_ln_scale, k_ln_scale, Wo):
    f16 = np.float16
    in_maps = []
    qsc = np.asarray(q_ln_scale, np.float32).reshape(Dh, 1)
    ksc = np.asarray(k_ln_scale, np.float32).reshape(Dh, 1)
    for c in range(N_CORES):
        b, g = c // 2, c % 2
        hs = slice(g * HG, (g + 1) * HG)
        in_maps.append({
            "xqT": np.ascontiguousarray(inputs_q[b].T).astype(f16),
            "xkT": np.ascontiguousarray(inputs_kv[b].T).astype(f16),
            "wq": Wq[:, hs, :].reshape(E, HG * Dh).astype(f16),
            "wk": Wk[:, hs, :].reshape(E, HG * Dh).astype(f16),
            "wv": Wv[:, hs, :].reshape(E, HG * Dh).astype(f16),
            "wo": np.ascontiguousarray(Wo[hs]).reshape(HG * Dh, E).astype(f16),
            "qs": qsc,
            "ks": ksc,
        })
    return in_maps


def kernel(inputs_q, inputs_kv, Wq, Wk, Wv, q_ln_scale, k_ln_scale, Wo,
           _trace=False, _trace_kwargs=None):
    from concourse.bass_utils import run_bass_kernel_spmd

    nc = _get_program()
    in_maps = _make_in_maps(np.asarray(inputs_q), np.asarray(inputs_kv),
                            np.asarray(Wq), np.asarray(Wk), np.asarray(Wv),
                            np.asarray(q_ln_scale), np.asarray(k_ln_scale),
                            np.asarray(Wo))
    res = run_bass_kernel_spmd(nc, in_maps, list(range(N_CORES)),
                               trace=_trace, **(_trace_kwargs or {}))
    outs = np.empty((B, S, E), np.float32)
    for b in range(B):
        outs[b] = res.results[2 * b]["out"] + res.results[2 * b + 1]["out"]
    if _trace:
        kernel.last_results = res
    return outs
